# revision 1
# baseline (speedup 1.0000x reference)
"""DynamicDistMatchingLoss — Bass/Tile kernel for TRN2, 8 NeuronCores SPMD.

Self-contained: takes FULL inputs (pred_dists (4,8,1048576) f32, means (4,8),
covs (4,8,8), indices (4,)), returns the full scalar loss (np.float32).

Math (per retained class chunk i with class id ci = indices[i] != 0):
  lp_j(x) = 0.5*||W_j x + v_j||^2 + c_j     W_j = chol(cov_j)^-1,
            v_j = -W_j mu_j,  c_j = 0.5*d*log(2pi) - sum(log(diag(L_j)))
  r(x)    = lp_ci(x) - log(1e-8 + sum_{j: idx[j]!=ci} e^lp_j + e^lp_ci)
  loss    = -(1/(C*N)) * sum_{i,n} r(x_n)

Sharding: data-parallel over N across the 8 cores (12 MiB/core of the 96 MiB
that matter; class-0 samples are never read). Tiny W/G/H/bias constants are
replicated. Each core returns 128x12 partial column sums; the host combines.

Device dataflow per core (48 tiles of 8192 samples; X layout: partition
p = d*16 + s, free = 512 sample-columns):
  stage1  PE    4x f32r matmul        Z_j = Wb_j @ X          (PSUM)
  square  ACT   Square(Z_j + v_j) -> SBUF bf16   (2 or 3 of 4 j's)
          DVE   tensor_scalar_add + bf16 self-mul (the rest)
  stage2  PE    4x bf16 matmul, PSUM-accumulated: M rows [64h+16j+s]
                hold the Mahalanobis terms for a PAIR of tiles
  exp     ACT   E = Exp(0.5*M + c_j) -> SBUF bf16 (one op per tile pair)
  fold    PE    S = H^T E: rows 32q+s = sum_j a_j e^lp (+target),
                rows 32q+16+s = e^lp_target; accumulated over the pair
  ln      ACT   Ln(2^-64*(S + 1e-8)) with accum_out -> one f32 column of
                lcols per 4-tile group (the 2^-64 pre-scale keeps the Ln
                LUT in its accurate domain; the shift cancels in the
                host's target-minus-sum combine)
Host: loss = -(1/(C*N)) * sum(lcols[target rows] - lcols[sum rows]).
"""
import numpy as np
import ml_dtypes
import bass_rust
import concourse.bass as bass
import concourse.tile as tile
from concourse import mybir

dt = mybir.dt
AF = mybir.ActivationFunctionType

LOG_2PI = float(np.log(2.0 * np.pi))
K, D = 4, 8
P = 128
SLOTS = 16
F = 512                       # samples per slot-column block
TILE_N = SLOTS * F            # 8192 samples per tile
LN_BATCH = 4                  # tiles per Ln group
GRP_N = TILE_N * LN_BATCH     # 32768 samples per group
LN_SCALE = float(2.0 ** -64)  # pre-scale for the Ln LUT domain
N_CORES = 8


def _legalize_multiwaits(nc):
    """This toolchain's walrus accepts at most one sem-wait per instruction;
    Tile's epilogue Drain carries several. Hoist extras onto NoOps."""
    n = 0
    for f in nc.m.functions:
        for bb in f.blocks:
            insts = list(bb.instructions)
            out = []
            changed = False
            for inst in insts:
                si = inst.sync_info
                if si is not None and len(si.on_wait) > 1:
                    waits = list(si.on_wait)
                    for w in waits[:-1]:
                        nop = bass_rust.InstNoOp(name=f"lgl_nop_{n}")
                        n += 1
                        nop.engine = inst.engine
                        nop.sync_info = bass_rust.SyncInfo(on_wait=[w],
                                                           on_update=[])
                        out.append(nop)
                    si.on_wait = [waits[-1]]
                    changed = True
                out.append(inst)
            if changed:
                bb.instructions = out
    return n


def _build_nc(n_chunks, npc):
    """Per-core Bass module; npc = samples per core per class chunk."""
    assert npc % GRP_N == 0
    ngrp_per_chunk = npc // GRP_N
    ngrp = n_chunks * ngrp_per_chunk

    nc = bass.Bass()
    xin = nc.declare_dram_parameter("xin", [n_chunks, P, npc // SLOTS],
                                    dt.float32r, isOutput=False)
    wstk = nc.declare_dram_parameter("wstk", [K, P, P], dt.float32r,
                                     isOutput=False)
    gmat_d = nc.declare_dram_parameter("gmat", [K, 2, P, P], dt.bfloat16,
                                       isOutput=False)
    hmat_d = nc.declare_dram_parameter("hmat", [n_chunks, 2, P, P],
                                       dt.bfloat16, isOutput=False)
    vb_d = nc.declare_dram_parameter("vb", [P, K], dt.float32, isOutput=False)
    cv_d = nc.declare_dram_parameter("cv", [P, 1], dt.float32, isOutput=False)
    outp = nc.declare_dram_parameter("outp", [P, ngrp], dt.float32,
                                     isOutput=True)

    with tile.TileContext(nc) as tc:
        with tc.tile_pool(name="const", bufs=1) as cpool, \
             tc.tile_pool(name="xload", bufs=3) as xpool, \
             tc.tile_pool(name="zsq", bufs=4) as zsqpool, \
             tc.tile_pool(name="epool", bufs=3) as epool, \
             tc.tile_pool(name="lnpool", bufs=2) as lnpool, \
             tc.tile_pool(name="zps", bufs=5, space="PSUM") as zpool, \
             tc.tile_pool(name="mps", bufs=2, space="PSUM") as mpool, \
             tc.tile_pool(name="sps", bufs=1, space="PSUM") as spool:

            wsb = cpool.tile([P, K * P], dt.float32r, name="wsb")
            nc.sync.dma_start(out=wsb[:].rearrange("p (k m) -> p k m", k=K),
                              in_=wstk[:, :, :].rearrange("k p m -> p k m"))
            gsb = cpool.tile([P, K * 2 * P], dt.bfloat16, name="gsb")
            nc.sync.dma_start(
                out=gsb[:].rearrange("p (k h m) -> p k h m", k=K, h=2),
                in_=gmat_d[:, :, :, :].rearrange("k h p m -> p k h m"))
            hsb = cpool.tile([P, n_chunks * 2 * P], dt.bfloat16, name="hsb")
            nc.sync.dma_start(
                out=hsb[:].rearrange("p (i u m) -> p i u m", i=n_chunks, u=2),
                in_=hmat_d[:, :, :, :].rearrange("i u p m -> p i u m"))
            vb = cpool.tile([P, K], dt.float32, name="vb")
            nc.sync.dma_start(out=vb[:], in_=vb_d[:, :])
            cv = cpool.tile([P, 1], dt.float32, name="cv")
            nc.sync.dma_start(out=cv[:], in_=cv_d[:, :])
            lcols = cpool.tile([P, ngrp], dt.float32, name="lcols")
            eps_t = cpool.tile([P, 1], dt.float32, name="eps_t")
            nc.vector.memset(eps_t[:], 1e-8 * LN_SCALE)

            for g in range(ngrp):
                i = g // ngrp_per_chunk
                g_in = g % ngrp_per_chunk
                s_ps = spool.tile([P, F], dt.float32, name="s_ps", tag="s_ps")
                xg = xpool.tile([P, LN_BATCH * F], dt.float32r, name="xg",
                                tag="xg")
                c0 = g_in * LN_BATCH * F
                nc.sync.dma_start(out=xg[:],
                                  in_=xin[i, :, c0:c0 + LN_BATCH * F])
                for pp in range(2):
                    m_ps = mpool.tile([P, F], dt.float32, name="m_ps",
                                      tag="m_ps")
                    for h in range(2):
                        q = 2 * pp + h
                        x_t = xg[:, q * F:(q + 1) * F]
                        for j in range(K):
                            z_ps = zpool.tile([P, F], dt.float32, name="z_ps",
                                              tag="z_ps")
                            nc.tensor.matmul(z_ps[:],
                                             lhsT=wsb[:, j * P:(j + 1) * P],
                                             rhs=x_t, start=True, stop=True)
                            zsq = zsqpool.tile([P, F], dt.bfloat16,
                                               name="zsq", tag=f"zsq{j}")
                            n_act = 2 + (q % 2)
                            if j < n_act:
                                nc.scalar.activation(zsq[:], z_ps[:],
                                                     AF.Square,
                                                     bias=vb[:, j:j + 1],
                                                     scale=1.0)
                            else:
                                zb = zsqpool.tile([P, F], dt.bfloat16,
                                                  name="zb", tag=f"zb{j}")
                                nc.vector.tensor_scalar_add(zb[:], z_ps[:],
                                                            vb[:, j:j + 1])
                                nc.vector.tensor_mul(zsq[:], zb[:], zb[:])
                            goff = (j * 2 + h) * P
                            nc.tensor.matmul(m_ps[:],
                                             lhsT=gsb[:, goff:goff + P],
                                             rhs=zsq[:],
                                             start=(h == 0 and j == 0),
                                             stop=(h == 1 and j == K - 1))
                    e_t = epool.tile([P, F], dt.bfloat16, name="e_t",
                                     tag="e_t")
                    nc.scalar.activation(e_t[:], m_ps[:], AF.Exp,
                                         bias=cv[:, 0:1], scale=0.5)
                    hoff = (i * 2 + pp) * P
                    nc.tensor.matmul(s_ps[:], lhsT=hsb[:, hoff:hoff + P],
                                     rhs=e_t[:], start=(pp == 0),
                                     stop=(pp == 1))
                ln_t = lnpool.tile([P, F], dt.bfloat16, name="ln_t",
                                   tag="ln_t")
                nc.scalar.activation(ln_t[:], s_ps[:], AF.Ln,
                                     bias=eps_t[:, 0:1], scale=LN_SCALE,
                                     accum_out=lcols[:, g:g + 1])
            nc.sync.dma_start(out=outp[:, :], in_=lcols[:])
    _legalize_multiwaits(nc)
    return nc


def _host_constants(means, covs, indices, n_chunks, chunk_classes):
    L = np.linalg.cholesky(np.asarray(covs, np.float64))
    Winv = np.stack([np.linalg.inv(L[j]) for j in range(K)])
    mu = np.asarray(means, np.float64)
    V = -np.einsum('jab,jb->ja', Winv, mu)
    hld = np.log(np.diagonal(L, axis1=1, axis2=2)).sum(axis=1)
    c = 0.5 * D * LOG_2PI - hld
    idx = [int(v) for v in np.asarray(indices)]

    Wb = np.zeros((K, P, P), np.float32)
    for dd in range(D):
        for dp in range(D):
            for s in range(SLOTS):
                Wb[:, dd * SLOTS + s, dp * SLOTS + s] = Winv[:, dp, dd]

    G = np.zeros((K, 2, P, P), np.float32)
    for j in range(K):
        for hh in range(2):
            for dd in range(D):
                for s in range(SLOTS):
                    G[j, hh, dd * SLOTS + s, 64 * hh + 16 * j + s] = 1.0

    H = np.zeros((n_chunks, 2, P, P), np.float32)
    for ci_pos, ipos in enumerate(chunk_classes):
        ci = idx[ipos]
        for p_ in range(2):
            for hh in range(2):
                for j in range(K):
                    a = ((1.0 if idx[j] != ci else 0.0) +
                         (1.0 if j == ci else 0.0))
                    for s in range(SLOTS):
                        H[ci_pos, p_, 64 * hh + 16 * j + s,
                          64 * p_ + 32 * hh + s] = a
                for s in range(SLOTS):
                    H[ci_pos, p_, 64 * hh + 16 * ci + s,
                      64 * p_ + 32 * hh + 16 + s] = 1.0

    vb = np.zeros((P, K), np.float32)
    for j in range(K):
        for dd in range(D):
            vb[dd * SLOTS:(dd + 1) * SLOTS, j] = V[j, dd]

    cvec = np.zeros((P, 1), np.float32)
    for hh in range(2):
        for j in range(K):
            cvec[64 * hh + 16 * j:64 * hh + 16 * (j + 1), 0] = c[j]
    return Wb, G, H, vb, cvec


_NC_CACHE = {}


def run_sharded(pred_dists, means, covs, indices, trace=False):
    """Returns (loss_f32, exec_time_ns_or_None)."""
    from concourse.bass_utils import run_bass_kernel_spmd

    pred_dists = np.asarray(pred_dists)
    idx = [int(v) for v in np.asarray(indices)]
    chunk_classes = [ipos for ipos, ci in enumerate(idx) if ci != 0]
    n_chunks = len(chunk_classes)
    if n_chunks == 0:
        return np.float32(0.0), None
    N = pred_dists.shape[2]
    npc = N // N_CORES
    assert npc % GRP_N == 0, (npc, GRP_N)
    ngrp_per_chunk = npc // GRP_N
    ngrp = n_chunks * ngrp_per_chunk

    Wb, G, H, vb, cvec = _host_constants(means, covs, indices, n_chunks,
                                         chunk_classes)
    key = (n_chunks, npc)
    if key not in _NC_CACHE:
        _NC_CACHE[key] = _build_nc(n_chunks, npc)
    nc = _NC_CACHE[key]

    in_maps = []
    for core in range(N_CORES):
        sl = pred_dists[chunk_classes, :, core * npc:(core + 1) * npc]
        # device layout: [chunk, d*16+s, t*F+f] = x[chunk, d, t*8192+s*512+f]
        sl = np.ascontiguousarray(
            sl.reshape(n_chunks, D, npc // TILE_N, SLOTS, F)
              .transpose(0, 1, 3, 2, 4)
              .reshape(n_chunks, P, npc // SLOTS)).astype(np.float32)
        in_maps.append({
            "xin": sl, "wstk": Wb,
            "gmat": G.astype(ml_dtypes.bfloat16),
            "hmat": H.astype(ml_dtypes.bfloat16),
            "vb": vb, "cv": cvec,
        })
    res = run_bass_kernel_spmd(nc, in_maps, list(range(N_CORES)), trace=trace)

    total = 0.0
    for core in range(N_CORES):
        lc = res.results[core]["outp"].astype(np.float64)
        for g in range(ngrp):
            for q in range(LN_BATCH):
                tgt = lc[32 * q + 16:32 * q + 32, g]
                smm = lc[32 * q:32 * q + 16, g]
                total += (tgt - smm).sum()
    loss = -total / (n_chunks * N)
    return np.float32(loss), res.exec_time_ns


def kernel(pred_dists, means, covs, indices):
    loss, _ = run_sharded(pred_dists, means, covs, indices, trace=False)
    return loss


# revision 2
# speedup vs baseline: 1.0109x; 1.0109x over previous
"""DynamicDistMatchingLoss — Bass/Tile kernel for TRN2, 8 NeuronCores SPMD.

Self-contained: takes FULL inputs (pred_dists (4,8,1048576) f32, means (4,8),
covs (4,8,8), indices (4,)), returns the full scalar loss (np.float32).

Math (per retained class chunk i with class id ci = indices[i] != 0):
  lp_j(x) = 0.5*||W_j x + v_j||^2 + c_j     W_j = chol(cov_j)^-1,
            v_j = -W_j mu_j,  c_j = 0.5*d*log(2pi) - sum(log(diag(L_j)))
  r(x)    = lp_ci(x) - log(1e-8 + sum_{j: idx[j]!=ci} e^lp_j + e^lp_ci)
  loss    = -(1/(C*N)) * sum_{i,n} r(x_n)

Sharding: data-parallel over N across the 8 cores (12 MiB/core of the 96 MiB
that matter; class-0 samples are never read). Tiny W/G/H/bias constants are
replicated. Each core returns 128x12 partial column sums; the host combines.

Device dataflow per core (48 tiles of 8192 samples; X layout: partition
p = d*16 + s, free = 512 sample-columns):
  stage1  PE    4x f32r matmul        Z_j = Wb_j @ X          (PSUM)
  square  ACT   Square(Z_j + v_j) -> SBUF bf16   (2 or 3 of 4 j's)
          DVE   tensor_scalar_add + bf16 self-mul (the rest)
  stage2  PE    4x bf16 matmul, PSUM-accumulated: M rows [64h+16j+s]
                hold the Mahalanobis terms for a PAIR of tiles
  exp     ACT   E = Exp(0.5*M + c_j) -> SBUF bf16 (one op per tile pair)
  fold    PE    S = H^T E: rows 32q+s = sum_j a_j e^lp (+target),
                rows 32q+16+s = e^lp_target; accumulated over the pair
  ln      ACT   Ln(2^-64*(S + 1e-8)) with accum_out -> one f32 column of
                lcols per 4-tile group (the 2^-64 pre-scale keeps the Ln
                LUT in its accurate domain; the shift cancels in the
                host's target-minus-sum combine)
Host: loss = -(1/(C*N)) * sum(lcols[target rows] - lcols[sum rows]).
"""
import numpy as np
import ml_dtypes
import bass_rust
import concourse.bass as bass
import concourse.tile as tile
from concourse import mybir

dt = mybir.dt
AF = mybir.ActivationFunctionType

LOG_2PI = float(np.log(2.0 * np.pi))
K, D = 4, 8
P = 128
SLOTS = 16
F = 512                       # samples per slot-column block
TILE_N = SLOTS * F            # 8192 samples per tile
LN_BATCH = 4                  # tiles per Ln group
GRP_N = TILE_N * LN_BATCH     # 32768 samples per group
LN_SCALE = float(2.0 ** -64)  # pre-scale for the Ln LUT domain
N_CORES = 8


def _legalize_multiwaits(nc):
    """This toolchain's walrus accepts at most one sem-wait per instruction;
    Tile's epilogue Drain carries several. Hoist extras onto NoOps."""
    n = 0
    for f in nc.m.functions:
        for bb in f.blocks:
            insts = list(bb.instructions)
            out = []
            changed = False
            for inst in insts:
                si = inst.sync_info
                if si is not None and len(si.on_wait) > 1:
                    waits = list(si.on_wait)
                    for w in waits[:-1]:
                        nop = bass_rust.InstNoOp(name=f"lgl_nop_{n}")
                        n += 1
                        nop.engine = inst.engine
                        nop.sync_info = bass_rust.SyncInfo(on_wait=[w],
                                                           on_update=[])
                        out.append(nop)
                    si.on_wait = [waits[-1]]
                    changed = True
                out.append(inst)
            if changed:
                bb.instructions = out
    return n


def _build_nc(n_chunks, npc):
    """Per-core Bass module; npc = samples per core per class chunk."""
    assert npc % GRP_N == 0
    ngrp_per_chunk = npc // GRP_N
    ngrp = n_chunks * ngrp_per_chunk

    nc = bass.Bass()
    xin = nc.declare_dram_parameter("xin", [n_chunks, P, npc // SLOTS],
                                    dt.float32r, isOutput=False)
    wstk = nc.declare_dram_parameter("wstk", [K, P, P], dt.float32r,
                                     isOutput=False)
    gmat_d = nc.declare_dram_parameter("gmat", [K, 2, P, P], dt.bfloat16,
                                       isOutput=False)
    hmat_d = nc.declare_dram_parameter("hmat", [n_chunks, 2, P, P],
                                       dt.bfloat16, isOutput=False)
    vb_d = nc.declare_dram_parameter("vb", [P, K], dt.float32, isOutput=False)
    cv_d = nc.declare_dram_parameter("cv", [P, 1], dt.float32, isOutput=False)
    outp = nc.declare_dram_parameter("outp", [P, ngrp], dt.float32,
                                     isOutput=True)

    with tile.TileContext(nc) as tc:
        with tc.tile_pool(name="const", bufs=1) as cpool, \
             tc.tile_pool(name="xload", bufs=4) as xpool, \
             tc.tile_pool(name="zsq", bufs=6) as zsqpool, \
             tc.tile_pool(name="epool", bufs=4) as epool, \
             tc.tile_pool(name="lnpool", bufs=2) as lnpool, \
             tc.tile_pool(name="zps", bufs=5, space="PSUM") as zpool, \
             tc.tile_pool(name="mps", bufs=2, space="PSUM") as mpool, \
             tc.tile_pool(name="sps", bufs=1, space="PSUM") as spool:

            wsb = cpool.tile([P, K * P], dt.float32r, name="wsb")
            nc.sync.dma_start(out=wsb[:].rearrange("p (k m) -> p k m", k=K),
                              in_=wstk[:, :, :].rearrange("k p m -> p k m"))
            gsb = cpool.tile([P, K * 2 * P], dt.bfloat16, name="gsb")
            nc.sync.dma_start(
                out=gsb[:].rearrange("p (k h m) -> p k h m", k=K, h=2),
                in_=gmat_d[:, :, :, :].rearrange("k h p m -> p k h m"))
            hsb = cpool.tile([P, n_chunks * 2 * P], dt.bfloat16, name="hsb")
            nc.sync.dma_start(
                out=hsb[:].rearrange("p (i u m) -> p i u m", i=n_chunks, u=2),
                in_=hmat_d[:, :, :, :].rearrange("i u p m -> p i u m"))
            vb = cpool.tile([P, K], dt.float32, name="vb")
            nc.sync.dma_start(out=vb[:], in_=vb_d[:, :])
            cv = cpool.tile([P, 1], dt.float32, name="cv")
            nc.sync.dma_start(out=cv[:], in_=cv_d[:, :])
            lcols = cpool.tile([P, ngrp], dt.float32, name="lcols")
            eps_t = cpool.tile([P, 1], dt.float32, name="eps_t")
            nc.vector.memset(eps_t[:], 1e-8 * LN_SCALE)

            for g in range(ngrp):
                i = g // ngrp_per_chunk
                g_in = g % ngrp_per_chunk
                s_ps = spool.tile([P, F], dt.float32, name="s_ps", tag="s_ps")
                xg = xpool.tile([P, LN_BATCH * F], dt.float32r, name="xg",
                                tag="xg")
                c0 = g_in * LN_BATCH * F
                nc.sync.dma_start(out=xg[:],
                                  in_=xin[i, :, c0:c0 + LN_BATCH * F])
                for pp in range(2):
                    m_ps = mpool.tile([P, F], dt.float32, name="m_ps",
                                      tag="m_ps")
                    for h in range(2):
                        q = 2 * pp + h
                        x_t = xg[:, q * F:(q + 1) * F]
                        for j in range(K):
                            z_ps = zpool.tile([P, F], dt.float32, name="z_ps",
                                              tag="z_ps")
                            nc.tensor.matmul(z_ps[:],
                                             lhsT=wsb[:, j * P:(j + 1) * P],
                                             rhs=x_t, start=True, stop=True)
                            zsq = zsqpool.tile([P, F], dt.bfloat16,
                                               name="zsq", tag=f"zsq{j}")
                            n_act = 2 + (q % 2)
                            if j < n_act:
                                nc.scalar.activation(zsq[:], z_ps[:],
                                                     AF.Square,
                                                     bias=vb[:, j:j + 1],
                                                     scale=1.0)
                            else:
                                zb = zsqpool.tile([P, F], dt.bfloat16,
                                                  name="zb", tag=f"zb{j}")
                                nc.vector.tensor_scalar_add(zb[:], z_ps[:],
                                                            vb[:, j:j + 1])
                                nc.vector.tensor_mul(zsq[:], zb[:], zb[:])
                            goff = (j * 2 + h) * P
                            nc.tensor.matmul(m_ps[:],
                                             lhsT=gsb[:, goff:goff + P],
                                             rhs=zsq[:],
                                             start=(h == 0 and j == 0),
                                             stop=(h == 1 and j == K - 1))
                    e_t = epool.tile([P, F], dt.bfloat16, name="e_t",
                                     tag="e_t")
                    nc.scalar.activation(e_t[:], m_ps[:], AF.Exp,
                                         bias=cv[:, 0:1], scale=0.5)
                    hoff = (i * 2 + pp) * P
                    nc.tensor.matmul(s_ps[:], lhsT=hsb[:, hoff:hoff + P],
                                     rhs=e_t[:], start=(pp == 0),
                                     stop=(pp == 1))
                ln_t = lnpool.tile([P, F], dt.bfloat16, name="ln_t",
                                   tag="ln_t")
                nc.scalar.activation(ln_t[:], s_ps[:], AF.Ln,
                                     bias=eps_t[:, 0:1], scale=LN_SCALE,
                                     accum_out=lcols[:, g:g + 1])
            nc.sync.dma_start(out=outp[:, :], in_=lcols[:])
    _legalize_multiwaits(nc)
    return nc


def _host_constants(means, covs, indices, n_chunks, chunk_classes):
    L = np.linalg.cholesky(np.asarray(covs, np.float64))
    Winv = np.stack([np.linalg.inv(L[j]) for j in range(K)])
    mu = np.asarray(means, np.float64)
    V = -np.einsum('jab,jb->ja', Winv, mu)
    hld = np.log(np.diagonal(L, axis1=1, axis2=2)).sum(axis=1)
    c = 0.5 * D * LOG_2PI - hld
    idx = [int(v) for v in np.asarray(indices)]

    Wb = np.zeros((K, P, P), np.float32)
    for dd in range(D):
        for dp in range(D):
            for s in range(SLOTS):
                Wb[:, dd * SLOTS + s, dp * SLOTS + s] = Winv[:, dp, dd]

    G = np.zeros((K, 2, P, P), np.float32)
    for j in range(K):
        for hh in range(2):
            for dd in range(D):
                for s in range(SLOTS):
                    G[j, hh, dd * SLOTS + s, 64 * hh + 16 * j + s] = 1.0

    H = np.zeros((n_chunks, 2, P, P), np.float32)
    for ci_pos, ipos in enumerate(chunk_classes):
        ci = idx[ipos]
        for p_ in range(2):
            for hh in range(2):
                for j in range(K):
                    a = ((1.0 if idx[j] != ci else 0.0) +
                         (1.0 if j == ci else 0.0))
                    for s in range(SLOTS):
                        H[ci_pos, p_, 64 * hh + 16 * j + s,
                          64 * p_ + 32 * hh + s] = a
                for s in range(SLOTS):
                    H[ci_pos, p_, 64 * hh + 16 * ci + s,
                      64 * p_ + 32 * hh + 16 + s] = 1.0

    vb = np.zeros((P, K), np.float32)
    for j in range(K):
        for dd in range(D):
            vb[dd * SLOTS:(dd + 1) * SLOTS, j] = V[j, dd]

    cvec = np.zeros((P, 1), np.float32)
    for hh in range(2):
        for j in range(K):
            cvec[64 * hh + 16 * j:64 * hh + 16 * (j + 1), 0] = c[j]
    return Wb, G, H, vb, cvec


_NC_CACHE = {}


def run_sharded(pred_dists, means, covs, indices, trace=False):
    """Returns (loss_f32, exec_time_ns_or_None)."""
    from concourse.bass_utils import run_bass_kernel_spmd

    pred_dists = np.asarray(pred_dists)
    idx = [int(v) for v in np.asarray(indices)]
    chunk_classes = [ipos for ipos, ci in enumerate(idx) if ci != 0]
    n_chunks = len(chunk_classes)
    if n_chunks == 0:
        return np.float32(0.0), None
    N = pred_dists.shape[2]
    npc = N // N_CORES
    assert npc % GRP_N == 0, (npc, GRP_N)
    ngrp_per_chunk = npc // GRP_N
    ngrp = n_chunks * ngrp_per_chunk

    Wb, G, H, vb, cvec = _host_constants(means, covs, indices, n_chunks,
                                         chunk_classes)
    key = (n_chunks, npc)
    if key not in _NC_CACHE:
        _NC_CACHE[key] = _build_nc(n_chunks, npc)
    nc = _NC_CACHE[key]

    in_maps = []
    for core in range(N_CORES):
        sl = pred_dists[chunk_classes, :, core * npc:(core + 1) * npc]
        # device layout: [chunk, d*16+s, t*F+f] = x[chunk, d, t*8192+s*512+f]
        sl = np.ascontiguousarray(
            sl.reshape(n_chunks, D, npc // TILE_N, SLOTS, F)
              .transpose(0, 1, 3, 2, 4)
              .reshape(n_chunks, P, npc // SLOTS)).astype(np.float32)
        in_maps.append({
            "xin": sl, "wstk": Wb,
            "gmat": G.astype(ml_dtypes.bfloat16),
            "hmat": H.astype(ml_dtypes.bfloat16),
            "vb": vb, "cv": cvec,
        })
    res = run_bass_kernel_spmd(nc, in_maps, list(range(N_CORES)), trace=trace)

    total = 0.0
    for core in range(N_CORES):
        lc = res.results[core]["outp"].astype(np.float64)
        for g in range(ngrp):
            for q in range(LN_BATCH):
                tgt = lc[32 * q + 16:32 * q + 32, g]
                smm = lc[32 * q:32 * q + 16, g]
                total += (tgt - smm).sum()
    loss = -total / (n_chunks * N)
    return np.float32(loss), res.exec_time_ns


def kernel(pred_dists, means, covs, indices):
    loss, _ = run_sharded(pred_dists, means, covs, indices, trace=False)
    return loss
